# revision 4
# baseline (speedup 1.0000x reference)
"""Trainium2 Bass kernel for the pixel-RNN (tanh RNN, T=784, H=512, B=256).

Strategy: data-parallel over batch (32 samples per core, 8 cores).
Per core, per time step:
  - PSUM split into two j-halves [32, 256] (separate banks).
  - x-term: K=2 matmul  [x_t; 1]^T @ [w_ih; b_ih+b_hh]  (start=True).
  - recurrence: 4 k-chunk matmuls per half, lhsT = hT chunk [128,32]
    (stationary), rhs = W_hh^T chunk [128,256] (moving).
  - tanh on ScalarE (PSUM -> SBUF h).
  - h -> hT via 4 PE transposes (PSUM) + VectorE copies (SBUF).

Precision plan: the first TB=752 steps run the matmuls in bf16 (PE streams
bf16 at 1 cycle/col vs fp32r's 2, and LDWEIGHTS halves too); the last 32
steps run in fp32r. The tanh map is contractive (~0.55/step), so the bf16
trajectory error (~6e-3 in logits) decays below 1e-5 after the fp32 tail —
verified against a numpy emulation of the full pipeline.

Final linear head (10 classes) on device; log-softmax / loss / argmax
on host (tiny [256,10] reduction).

A (self-loading) matmul can carry at most ONE sync wait in codegen,
and each dma_start lands on its own DMA queue (own semaphore). So after the
constant DMAs, one tiny "gate" matmul per DMA absorbs that queue's semaphore
into the PE's observed clock; every later matmul then needs at most one wait.
"""

import sys

if "/opt/trn_rl_repo" not in sys.path:
    sys.path.insert(0, "/opt/trn_rl_repo")

import numpy as np

B, T, H, NCLS = 256, 784, 512, 10
NCORES = 8
BC = B // NCORES   # 32 samples per core
KC = H // 128      # 4 contraction chunks
JH = H // 2        # 256, j-half width
TAIL = 32          # trailing steps in fp32r
TB = T - TAIL      # leading steps in bf16

_BUILD_CACHE = {}


def _build(split_waits=True):
    """Build the Bass module (single program, run SPMD on 8 cores)."""
    import concourse.bass as bass
    import concourse.mybir as mybir
    from concourse import tile

    f32 = mybir.dt.float32
    f32r = mybir.dt.float32r
    bf16 = mybir.dt.bfloat16
    Tanh = mybir.ActivationFunctionType.Tanh

    nc = bass.Bass(
        "TRN2",
        target_bir_lowering=False,
        debug=False,
        enable_asserts=False,
        num_devices=NCORES,
    )

    d_xTb = nc.dram_tensor("xTb", (2, TB * BC), bf16, kind="ExternalInput").ap()
    d_xTf = nc.dram_tensor("xTf", (2, TAIL * BC), f32r, kind="ExternalInput").ap()
    d_wb = nc.dram_tensor("wihb_b", (2, H), bf16, kind="ExternalInput").ap()
    d_wf = nc.dram_tensor("wihb_f", (2, H), f32r, kind="ExternalInput").ap()
    d_WTb = nc.dram_tensor("WTb", (128, KC * H), bf16, kind="ExternalInput").ap()
    d_WTf = nc.dram_tensor("WTf", (128, KC * H), f32r, kind="ExternalInput").ap()
    d_lWT = nc.dram_tensor("lWT", (128, KC * NCLS), f32r, kind="ExternalInput").ap()
    d_idb = nc.dram_tensor("identb", (32, 32), bf16, kind="ExternalInput").ap()
    d_idf = nc.dram_tensor("identf", (32, 32), f32, kind="ExternalInput").ap()
    d_out = nc.dram_tensor("logitsT", (NCLS, BC), f32, kind="ExternalOutput").ap()

    with tile.TileContext(nc) as tc:
        with (
            tc.tile_pool(name="const", bufs=1) as cpool,
            tc.tile_pool(name="ps", bufs=1, space="PSUM") as ppool,
        ):
            xTb_sb = cpool.tile([2, TB * BC], bf16, tag="xTb")
            xTf_sb = cpool.tile([2, TAIL * BC], f32r, tag="xTf")
            wb_sb = cpool.tile([2, H], bf16, tag="wb")
            wf_sb = cpool.tile([2, H], f32r, tag="wf")
            WTb_sb = cpool.tile([128, KC * H], bf16, tag="WTb")
            WTf_sb = cpool.tile([128, KC * H], f32r, tag="WTf")
            lWT_sb = cpool.tile([128, KC * NCLS], f32r, tag="lWT")
            idb_sb = cpool.tile([32, 32], bf16, tag="identb")
            idf_sb = cpool.tile([32, 32], f32, tag="identf")
            out_sb = cpool.tile([NCLS, BC], f32, tag="out")

            # ping-pong working set: allocated once -> no tile-slot releases,
            # so every hot-path instruction needs at most one sync wait.
            hhb = [cpool.tile([BC, H], bf16, tag=f"hb{p}", name=f"hb{p}")
                   for p in range(2)]
            hTb = [cpool.tile([128, KC * BC], bf16, tag=f"hTb{p}", name=f"hTb{p}")
                   for p in range(2)]
            hhf = [cpool.tile([BC, H], f32, tag=f"hf{p}", name=f"hf{p}")
                   for p in range(2)]
            hTf = [cpool.tile([128, KC * BC], f32r, tag=f"hTf{p}", name=f"hTf{p}")
                   for p in range(2)]
            # PSUM: per parity, 2 accumulators [32, 256] (separate banks)
            # shared by both phases, plus transpose banks per dtype.
            ph = [[ppool.tile([BC, JH], f32, tag=f"ph{p}{i}", name=f"ph{p}{i}")
                   for i in range(2)] for p in range(2)]
            # PSUM is allocated at bank granularity (8 banks total): share the
            # transpose banks between phases via a bf16 bitcast view.
            ptf = [[ppool.tile([128, 2 * BC], f32, tag=f"ptf{p}{i}",
                               name=f"ptf{p}{i}") for i in range(2)]
                   for p in range(2)]
            ptb = [[ptf[p][i][:, :].bitcast(bf16) for i in range(2)]
                   for p in range(2)]

            dmas = [
                (xTb_sb, d_xTb), (xTf_sb, d_xTf), (wb_sb, d_wb), (wf_sb, d_wf),
                (lWT_sb, d_lWT), (idb_sb, d_idb), (idf_sb, d_idf),
            ]
            for sb, dr in dmas:
                nc.sync.dma_start(out=sb[:, :], in_=dr)
            for kc in range(KC):
                nc.sync.dma_start(out=WTb_sb[:, kc * H:(kc + 1) * H],
                                  in_=d_WTb[:, kc * H:(kc + 1) * H])
                nc.sync.dma_start(out=WTf_sb[:, kc * H:(kc + 1) * H],
                                  in_=d_WTf[:, kc * H:(kc + 1) * H])

            # gate matmuls: one per DMA, each absorbing one queue semaphore
            # into the PE's observed clock (results discarded)
            gates = [
                (xTb_sb[0:2, 0:BC], xTb_sb[0:2, 0:JH]),
                (xTf_sb[0:2, 0:BC], xTf_sb[0:2, 0:JH]),
                (wb_sb[0:2, 0:BC], wb_sb[0:2, 0:JH]),
                (wf_sb[0:2, 0:BC], wf_sb[0:2, 0:JH]),
                (lWT_sb[:, 0:32], lWT_sb[:, 0:KC * NCLS]),
            ]
            for kc in range(KC):
                gates.append(
                    (WTb_sb[:, kc * H:kc * H + BC],
                     WTb_sb[:, kc * H:kc * H + JH]))
                gates.append(
                    (WTf_sb[:, kc * H:kc * H + BC],
                     WTf_sb[:, kc * H:kc * H + JH]))
            for glhs, grhs in gates:
                nc.tensor.matmul(ph[0][0][:, 0:grhs.shape[-1]], glhs, grhs,
                                 start=True, stop=True)
            nc.tensor.matmul(ph[0][0][0:32, 0:32], idb_sb[:, 0:32],
                             idb_sb[:, :], start=True, stop=True)
            nc.tensor.matmul(ph[0][0][0:32, 0:32], idf_sb[:, 0:32],
                             idf_sb[:, :], start=True, stop=True)

            for t in range(T):
                p, q = t % 2, 1 - (t % 2)
                first = t == 0
                bstep = t < TB           # matmuls in bf16 this step?
                bout = t < TB - 1        # h written as bf16 this step?
                if bstep:
                    xlhs = xTb_sb[0:2, t * BC:(t + 1) * BC]
                    wsb, WTsb, hTq = wb_sb, WTb_sb, hTb[q]
                else:
                    tt = t - TB
                    xlhs = xTf_sb[0:2, tt * BC:(tt + 1) * BC]
                    wsb, WTsb, hTq = wf_sb, WTf_sb, hTf[q]
                # emit half0's x-term + full contraction before touching
                # half1: psum half0 completes one matmul-slot earlier, so the
                # tanh0 -> transpose -> copy0 chain (which feeds the next
                # step's first matmuls) starts earlier.
                for half in range(2):
                    nc.tensor.matmul(
                        ph[p][half][:, :],
                        xlhs,
                        wsb[0:2, half * JH:(half + 1) * JH],
                        start=True,
                        stop=first,
                    )
                    if not first:
                        for kc in range(KC):
                            nc.tensor.matmul(
                                ph[p][half][:, :],
                                hTq[:, kc * BC:(kc + 1) * BC],
                                WTsb[
                                    :, kc * H + half * JH: kc * H + (half + 1) * JH
                                ],
                                start=False,
                                stop=(kc == KC - 1),
                            )

                hh = hhb[p] if bout else hhf[p]
                hT = hTb[p] if bout else hTf[p]
                pt = ptb[p] if bout else ptf[p]
                id_sb = idb_sb if bout else idf_sb
                for half in range(2):
                    nc.scalar.activation(
                        hh[:, half * JH:(half + 1) * JH], ph[p][half][:, :],
                        Tanh,
                    )

                for i in range(2):
                    for j in range(2):
                        kc = 2 * i + j
                        nc.tensor.transpose(
                            pt[i][:, j * BC:(j + 1) * BC],
                            hh[0:BC, kc * 128:(kc + 1) * 128],
                            id_sb[:, :],
                        )
                    nc.vector.tensor_copy(
                        hT[:, i * 2 * BC:(i + 1) * 2 * BC], pt[i][:, 0:2 * BC]
                    )

            # final linear head: logitsT[c, b] = sum_j lin_W[c, j] h[b, j]
            pl = (T - 1) % 2
            pL = ph[1 - pl][0]
            for kc in range(KC):
                nc.tensor.matmul(
                    pL[0:NCLS, 0:BC],
                    lWT_sb[:, kc * NCLS:(kc + 1) * NCLS],
                    hTf[pl][:, kc * BC:(kc + 1) * BC],
                    start=(kc == 0),
                    stop=(kc == KC - 1),
                )
            nc.vector.tensor_copy(out_sb[:, :], pL[0:NCLS, 0:BC])
            nc.sync.dma_start(out=d_out, in_=out_sb[:, :])

    if split_waits:
        _split_multi_waits(nc, mybir)
    return nc


def _split_multi_waits(nc, mybir):
    """Walrus can pack only one sync wait into a HW instruction. Move any
    extra waits onto same-engine NoOps inserted right before (the engine's
    sequencer executes them in order, so semantics are unchanged)."""
    nid = 0
    for b in nc.m.functions[0].blocks:
        out = []
        changed = False
        for ins in b.instructions:
            si = getattr(ins, "sync_info", None)
            ws = list(getattr(si, "on_wait", []) or []) if si else []
            if len(ws) > 1:
                for w in ws[:-1]:
                    nid += 1
                    out.append(mybir.InstNoOp(
                        name=f"I-wsplit-{nid}",
                        engine=ins.engine,
                        sync_info=mybir.SyncInfo(on_wait=[w], on_update=[]),
                    ))
                ins.sync_info = mybir.SyncInfo(
                    on_wait=[ws[-1]], on_update=list(si.on_update or [])
                )
                changed = True
            out.append(ins)
        if changed:
            b.instructions = out
    return nc


def _pack_inputs(inputs, order, W_ih, b_ih, W_hh, b_hh, lin_W):
    """Host-side shard packing: returns in_maps list (one dict per core)."""
    import ml_dtypes

    bf = ml_dtypes.bfloat16
    x = np.asarray(inputs, np.float32)[:, np.asarray(order, np.int64)]
    wihb = np.stack(
        [np.asarray(W_ih, np.float32)[:, 0],
         np.asarray(b_ih, np.float32) + np.asarray(b_hh, np.float32)]
    )  # [2, H]
    WT = np.ascontiguousarray(
        np.asarray(W_hh, np.float32).T.reshape(KC, 128, H)
        .transpose(1, 0, 2).reshape(128, KC * H)
    )
    lWT = np.ascontiguousarray(
        np.asarray(lin_W, np.float32).T.reshape(KC, 128, NCLS)
        .transpose(1, 0, 2).reshape(128, KC * NCLS)
    )
    ident = np.eye(32, dtype=np.float32)

    WTb = WT.astype(bf)
    wb = wihb.astype(bf)
    idb = ident.astype(bf)

    in_maps = []
    for c in range(NCORES):
        xc = x[c * BC:(c + 1) * BC]  # [BC, T]
        xT = np.ones((2, T * BC), np.float32)
        xT[0] = xc.T.reshape(-1)
        in_maps.append({
            "xTb": xT[:, :TB * BC].astype(bf),
            "xTf": np.ascontiguousarray(xT[:, TB * BC:]),
            "wihb_b": wb, "wihb_f": wihb,
            "WTb": WTb, "WTf": WT,
            "lWT": lWT, "identb": idb, "identf": ident,
        })
    return in_maps


def _run(inputs, y, order, W_ih, b_ih, W_hh, b_hh, lin_W, lin_b, trace=False):
    from concourse import bass_utils

    key = "k"
    if key not in _BUILD_CACHE:
        _BUILD_CACHE[key] = _build()
    nc = _BUILD_CACHE[key]

    in_maps = _pack_inputs(inputs, order, W_ih, b_ih, W_hh, b_hh, lin_W)
    res = bass_utils.run_bass_kernel_spmd(
        nc, in_maps, core_ids=list(range(NCORES)), trace=trace
    )

    logits = np.empty((B, NCLS), np.float32)
    for c in range(NCORES):
        logits[c * BC:(c + 1) * BC] = res.results[c]["logitsT"].T
    logits = logits + np.asarray(lin_b, np.float32)[None, :]

    yv = np.asarray(y).astype(np.int64)
    m = logits.max(axis=1, keepdims=True)
    logp = logits - (np.log(np.exp(logits - m).sum(axis=1, keepdims=True)) + m)
    loss = np.float32(-logp[np.arange(B), yv].mean())
    correct = np.int32((logits.argmax(axis=1) == yv).sum())
    return (loss, correct), res


def kernel(inputs, y, order, W_ih, b_ih, W_hh, b_hh, lin_W, lin_b):
    out, _ = _run(inputs, y, order, W_ih, b_ih, W_hh, b_hh, lin_W, lin_b)
    return out
